# revision 1
# baseline (speedup 1.0000x reference)
"""2-layer GCN on 8 trn2 NeuronCores (Bass/Tile, SPMD).

Strategy (dst-sharded gather aggregation):
- Host: add self-loops, compute dinv, sort nodes by in-degree, serpentine-
  assign 128-node blocks to the 8 cores, build per-core edge streams
  (sorted by dst block, split into int16-addressable table windows).
- Device launch per layer:
    phase 1: table[tau(n)] = (x @ W)[n] rows (bf16, 256B) where tau is a
             partition-major swizzle that makes table writes contiguous.
    phase 2: per group of dst blocks: dma_gather edge messages, scale by
             edge norm dinv[src]*dinv[dst], selection-matrix matmuls
             accumulate per-block aggregates in PSUM, bias(+relu) -> zT.
- Host between layers reassembles h_act in node order; final unpermute.
"""
import math
import numpy as np
import ml_dtypes

BF16 = ml_dtypes.bfloat16
NCORES = 8
P = 128

N_NODES = 100000
N_FEAT = 128
HIDDEN = 64
N_CLASSES = 40

# ---------------------------------------------------------------------------
# Tile patch: this container's walrus supports only ONE sem-wait per
# instruction. Split Tile's exit-drain waits and any multi-wait instruction
# across single-wait same-engine NoOps (identical semantics: the sequencer
# blocks on the nops first).
# ---------------------------------------------------------------------------
_patched = False


def _apply_tile_patch():
    global _patched
    if _patched:
        return
    _patched = True
    import concourse.tile as tile
    import concourse.mybir as mybir
    from concourse.vector_clock import VectorClock
    from concourse.tile_sem_assignment import N_PROCS

    def _split_drain_and_barrier(self, tick_clock, wait_clock):
        nc = self.nc
        gc = tick_clock.global_clock
        procs = [p for p in range(N_PROCS) if gc[p] > 0]
        for i, pr in enumerate(procs):
            sub = VectorClock([gc[p] if p == pr else 0 for p in range(N_PROCS)])
            ins = nc.sync.nop(nofuse=True, hint=f"drain_split_{i}")
            wait_clock.add_sem_waits(ins.ins, tile.ScopedClock({None: sub}))
        nc.sync.drain()
        nc.all_engine_barrier()
        assert self.sems is not None
        popped = nc._tile_sem_poison_stack.pop()
        assert popped is self._sem_poison
        nc.clear_and_free_semaphores(list(self.sems.allocated().values()))
        nc.all_engine_barrier()

    _orig = tile.TileContext._commit_and_lower
    _special = (
        mybir.BassTileCriticalSection,
        tile.BassTileBranchHintPlaceholder,
        tile.BassTileRelease,
    )

    def _split_commit_and_lower(self, inst, original_block, old_bb_map, bb_to_exit_bb):
        si = inst.sync_info
        if (
            si is not None
            and len(si.on_wait) > 1
            and inst.engine is not None
            and not isinstance(inst, _special)
        ):
            waits = list(si.on_wait)
            for w in waits[:-1]:
                nop = mybir.InstNoOp(
                    name=self.nc.get_next_instruction_name(),
                    engine=inst.engine,
                    ins=[],
                    outs=[],
                    bass_nofuse=True,
                    sync_info=mybir.SyncInfo(on_wait=[w], on_update=[]),
                )
                _orig(self, nop, original_block, old_bb_map, bb_to_exit_bb)
            inst.sync_info = mybir.SyncInfo(
                on_wait=waits[-1:], on_update=list(si.on_update)
            )
        return _orig(self, inst, original_block, old_bb_map, bb_to_exit_bb)

    tile.TileContext._drain_and_barrier = _split_drain_and_barrier
    tile.TileContext._commit_and_lower = _split_commit_and_lower


class Cfg:
    def __init__(self, n, f_in, hid, ncls, winrows=32768, target_cols=26):
        self.N = n
        self.F = f_in
        self.H = hid
        self.C = ncls
        self.WINROWS = winrows
        self.TARGET_COLS = target_cols
        self.NCHUNK = math.ceil(n / P)
        self.NPAD = P * self.NCHUNK
        self.NBT = NCORES * math.ceil(self.NCHUNK / NCORES)
        self.NB = self.NBT // NCORES
        self.NWIN = math.ceil(self.NPAD / self.WINROWS)


class Struct:
    pass


def prep(cfg, edge_index):
    N = cfg.N
    ei = np.asarray(edge_index)
    src = np.concatenate([ei[0].astype(np.int64), np.arange(N, dtype=np.int64)])
    dst = np.concatenate([ei[1].astype(np.int64), np.arange(N, dtype=np.int64)])
    deg = np.bincount(dst, minlength=N).astype(np.float64)
    dinv = np.where(deg > 0, 1.0 / np.sqrt(deg), 0.0).astype(np.float32)

    order = np.argsort(-deg, kind="stable")
    invorder = np.empty(N, np.int64)
    invorder[order] = np.arange(N)

    b_all = np.arange(cfg.NBT)
    s_all = b_all // NCORES
    i_all = b_all % NCORES
    core_of_block = np.where(s_all % 2 == 0, i_all, (NCORES - 1) - i_all)

    pos_d = invorder[dst]
    b_d = pos_d // P
    core_e = core_of_block[b_d]
    lb_e = b_d // NCORES
    dstl_e = (pos_d % P).astype(np.int64)
    tau = (src % P) * cfg.NCHUNK + src // P
    w_e = tau // cfg.WINROWS
    loc_e = (tau % cfg.WINROWS).astype(np.int64)
    norm_e = (dinv[src] * dinv[dst]).astype(np.float32)

    NB, NWIN = cfg.NB, cfg.NWIN
    key = (core_e * NB + lb_e) * NWIN + w_e
    sortidx = np.argsort(key, kind="stable")
    counts = np.bincount(key, minlength=NCORES * NB * NWIN).reshape(NCORES, NB, NWIN)
    C = np.ceil(counts.max(axis=0) / P).astype(np.int64)
    C[:, 0] = np.maximum(C[:, 0], 1)

    st = Struct()
    st.C = C
    st.LEN_W = (C.sum(axis=0) * P).astype(np.int64)
    st.LWT = int(st.LEN_W.sum()) // 16

    groups = []
    cur = []
    cur_cols = np.zeros(NWIN, np.int64)
    for lb in range(NB):
        if cur and np.any(cur_cols + C[lb] > cfg.TARGET_COLS):
            groups.append(cur)
            cur = []
            cur_cols = np.zeros(NWIN, np.int64)
        cur.append(lb)
        cur_cols += C[lb]
    if cur:
        groups.append(cur)
    st.groups = groups
    st.NG = len(groups)

    soff = np.zeros((NB, NWIN), np.int64)
    for w in range(NWIN):
        soff[:, w] = np.concatenate([[0], np.cumsum(C[:-1, w] * P)])
    wbase = np.concatenate([[0], np.cumsum(st.LEN_W)])

    colbase_gw = {}
    col_of_lbw = np.full((NB, NWIN), -1, np.int64)
    col = 0
    for g, lbs in enumerate(groups):
        for w in range(NWIN):
            colbase_gw[(g, w)] = col
            for lb in lbs:
                col_of_lbw[lb, w] = col
                col += int(C[lb, w])
    st.CT = col
    st.colbase_gw = colbase_gw

    edge_rank = np.empty(len(src), np.int64)
    ck = key[sortidx]
    seg_start = np.concatenate(
        [[0], np.cumsum(np.bincount(ck, minlength=NCORES * NB * NWIN))]
    )[:-1]
    edge_rank[sortidx] = np.arange(len(src)) - seg_start[ck]

    idx16 = np.zeros((NCORES, 16, st.LWT), np.int16)
    dstl_a = np.full((NCORES, P, st.CT), -1.0, np.float32)
    norm_a = np.zeros((NCORES, P, st.CT), np.float32)

    epos = soff[lb_e, w_e] + edge_rank
    gpos = wbase[w_e] + epos
    idx16[core_e, gpos % 16, gpos // 16] = loc_e.astype(np.int16)
    ecol = col_of_lbw[lb_e, w_e] + (epos - soff[lb_e, w_e]) // P
    epart = gpos % P
    dstl_a[core_e, epart, ecol] = dstl_e.astype(np.float32)
    norm_a[core_e, epart, ecol] = norm_e

    st.idx16 = idx16
    st.dstl = dstl_a.astype(BF16)
    st.norm = norm_a.astype(BF16)

    st.lb_chunks = []
    for g, lbs in enumerate(groups):
        for lb in lbs:
            ch = []
            for w in range(NWIN):
                base_stage = col_of_lbw[lb, w] - colbase_gw[(g, w)]
                for j in range(int(C[lb, w])):
                    ch.append((w, int(col_of_lbw[lb, w] + j), int(base_stage + j)))
            st.lb_chunks.append((g, lb, ch))

    st.gcall = {}
    for g, lbs in enumerate(groups):
        for w in range(NWIN):
            cols = sum(int(C[lb, w]) for lb in lbs)
            if cols == 0:
                continue
            el0 = int(wbase[w] + soff[lbs[0], w])
            st.gcall[(g, w)] = (cols, el0 // 16)
    st.MAXCOLS = [
        max((st.gcall[(g, w)][0] for g in range(st.NG) if (g, w) in st.gcall),
            default=0)
        for w in range(NWIN)
    ]
    st.winrows = [
        (w * cfg.WINROWS, min(cfg.WINROWS, cfg.NPAD - w * cfg.WINROWS))
        for w in range(NWIN)
    ]
    st.order = order
    iota = np.broadcast_to(np.arange(P, dtype=np.float32), (P, P))
    st.iota = np.ascontiguousarray(iota).astype(BF16)
    st.struct_key = (
        cfg.N, cfg.WINROWS, cfg.TARGET_COLS, st.CT, st.LWT,
        hash(st.C.tobytes()),
    )
    return st


def build_layer(cfg, st, kin, km, relu, out_f32):
    _apply_tile_patch()
    import concourse.bacc as bacc
    import concourse.mybir as mybir
    import concourse.tile as tile
    from contextlib import ExitStack

    nc = bacc.Bacc("TRN2", target_bir_lowering=False, debug=False,
                   num_devices=NCORES)
    dt = mybir.dt
    NPAD, NCHUNK, NB, NWIN = cfg.NPAD, cfg.NCHUNK, cfg.NB, cfg.NWIN

    xT = nc.declare_dram_parameter("xT", [kin, NPAD], dt.bfloat16, isOutput=False)
    W = nc.declare_dram_parameter("W", [kin, km], dt.bfloat16, isOutput=False)
    bias = nc.declare_dram_parameter("bias", [km, 1], dt.float32, isOutput=False)
    iota = nc.declare_dram_parameter("iota", [P, P], dt.bfloat16, isOutput=False)
    idx16 = nc.declare_dram_parameter("idx16", [16, st.LWT], dt.int16, isOutput=False)
    dstl = nc.declare_dram_parameter("dstl", [P, st.CT], dt.bfloat16, isOutput=False)
    norm = nc.declare_dram_parameter("norm", [P, st.CT], dt.bfloat16, isOutput=False)
    zdt = dt.float32 if out_f32 else dt.bfloat16
    zT = nc.declare_dram_parameter("zT", [km, NB * P], zdt, isOutput=True)
    tab = nc.dram_tensor("tab", [NPAD, P], dt.bfloat16)
    tabv = tab[:].rearrange("(q c) e -> q (c e)", q=P, c=NCHUNK)

    with tile.TileContext(nc) as tc, ExitStack() as ctx:
        cpool = ctx.enter_context(tc.tile_pool(name="consts", bufs=1))
        Wt = cpool.tile([kin, km], dt.bfloat16)
        nc.sync.dma_start(Wt[:], W[:])
        bias_t = cpool.tile([km, 1], dt.float32)
        nc.sync.dma_start(bias_t[:], bias[:])
        iota_t = cpool.tile([P, P], dt.bfloat16)
        nc.sync.dma_start(iota_t[:], iota[:])

        # phase 1: message table
        with tc.tile_pool(name="xt", bufs=3) as xpool, \
             tc.tile_pool(name="tstag", bufs=3) as spool, \
             tc.tile_pool(name="psum1", bufs=8, space="PSUM") as pp1:
            for c0 in range(0, NCHUNK, 8):
                nj = min(8, NCHUNK - c0)
                xt = xpool.tile([kin, 8 * P], dt.bfloat16, tag="xt")
                nc.sync.dma_start(xt[:, :nj * P], xT[:, c0 * P:(c0 + nj) * P])
                stag = spool.tile([P, 8 * P], dt.bfloat16, tag="tstag")
                nc.vector.memset(stag[:], 0.0)
                for j in range(nj):
                    ps = pp1.tile([P, km], dt.float32, tag="ps1")
                    nc.tensor.matmul(out=ps[:], lhsT=xt[:, j * P:(j + 1) * P],
                                     rhs=Wt[:], start=True, stop=True)
                    nc.vector.tensor_copy(stag[:, j * P:j * P + km], ps[:])
                nc.sync.dma_start(tabv[:, c0 * P:(c0 + nj) * P], stag[:, :nj * P])

        # phase 2: gather + aggregate
        with tc.tile_pool(name="idx", bufs=1) as ipool, \
             tc.tile_pool(name="meta", bufs=1) as mpool, \
             tc.tile_pool(name="stg", bufs=2) as gpool, \
             tc.tile_pool(name="S", bufs=4) as Spool, \
             tc.tile_pool(name="psum2", bufs=8, space="PSUM") as pp2, \
             tc.tile_pool(name="zst", bufs=2) as zpool:
            idx_sb = ipool.tile([P, st.LWT], dt.int16)
            for k in range(8):
                nc.sync.dma_start(idx_sb[16 * k:16 * (k + 1), :], idx16[:])
            dstl_sb = mpool.tile([P, st.CT], dt.bfloat16)
            nc.sync.dma_start(dstl_sb[:], dstl[:])
            norm_sb = mpool.tile([P, st.CT], dt.bfloat16)
            nc.sync.dma_start(norm_sb[:], norm[:])

            lbi = 0
            for g, lbs in enumerate(st.groups):
                stages = {}
                for w in range(NWIN):
                    if (g, w) not in st.gcall:
                        continue
                    cols, icol0 = st.gcall[(g, w)]
                    stg = gpool.tile([P, max(st.MAXCOLS[w], 1), P], dt.bfloat16,
                                     tag=f"stg{w}")
                    w0, wr = st.winrows[w]
                    nc.gpsimd.dma_gather(
                        out_ap=stg[:, :cols, :],
                        in_ap=tab[w0:w0 + wr, :],
                        idxs_ap=idx_sb[:, icol0:icol0 + cols * 8],
                        num_idxs=cols * P,
                        num_idxs_reg=cols * P,
                        elem_size=P,
                        single_packet=(cols * P <= 1024),
                    )
                    cb = st.colbase_gw[(g, w)]
                    nc.vector.tensor_tensor(
                        out=stg[:, :cols, :km],
                        in0=stg[:, :cols, :km],
                        in1=norm_sb[:, cb:cb + cols].to_broadcast([P, cols, km]),
                        op=mybir.AluOpType.mult,
                    )
                    stages[w] = stg

                gc0 = st.colbase_gw[(g, 0)]
                gc1 = gc0 + sum(st.gcall.get((g, w), (0, 0))[0] for w in range(NWIN))
                Stiles = {}
                for s0 in range(gc0, gc1, 8):
                    nb8 = min(8, gc1 - s0)
                    S = Spool.tile([P, 8, P], dt.bfloat16, tag="S")
                    nc.vector.tensor_tensor(
                        out=S[:, :nb8, :],
                        in0=iota_t[:, None, :].to_broadcast([P, nb8, P]),
                        in1=dstl_sb[:, s0:s0 + nb8].to_broadcast([P, nb8, P]),
                        op=mybir.AluOpType.is_equal,
                    )
                    Stiles[s0] = S

                zs = zpool.tile([km, len(lbs), P], zdt, tag="zs")
                for bi, lb in enumerate(lbs):
                    gg, lb2, chunks = st.lb_chunks[lbi]
                    assert gg == g and lb2 == lb
                    lbi += 1
                    ps = pp2.tile([km, P], dt.float32, tag="ps2")
                    nmm = len(chunks)
                    for t, (w, gcol, scol) in enumerate(chunks):
                        S = Stiles[gc0 + ((gcol - gc0) // 8) * 8]
                        nc.tensor.matmul(
                            out=ps[:],
                            lhsT=stages[w][:, scol, :km],
                            rhs=S[:, (gcol - gc0) % 8, :],
                            start=(t == 0), stop=(t == nmm - 1),
                        )
                    nc.scalar.activation(
                        out=zs[:, bi, :], in_=ps[:],
                        func=(mybir.ActivationFunctionType.Relu if relu
                              else mybir.ActivationFunctionType.Identity),
                        bias=bias_t[:],
                    )
                lb0 = lbs[0]
                nc.sync.dma_start(
                    zT[:, lb0 * P:(lb0 + len(lbs)) * P],
                    zs[:].rearrange("k b p -> k (b p)"),
                )
    nc.compile()
    return nc


SHARED_INPUTS = ("xT", "W", "bias", "iota")


def make_runner(nc):
    """jit-compiled 8-core runner with replicated shared inputs.

    Returns (fn, in_names, out_names, out_avals): fn takes per-input jax
    arrays (shared ones un-stacked, per-core ones stacked on axis 0) plus
    stacked zero output buffers, returns stacked outputs.
    """
    import jax
    import concourse.mybir as mybir
    from concourse import bass2jax
    from jax.sharding import Mesh, PartitionSpec
    from jax.experimental.shard_map import shard_map

    bass2jax.install_neuronx_cc_hook()
    partition_name = (
        nc.partition_id_tensor.name if nc.partition_id_tensor else None
    )
    in_names, out_names, out_avals, zero_outs = [], [], [], []
    for alloc in nc.m.functions[0].allocations:
        if not isinstance(alloc, mybir.MemoryLocationSet):
            continue
        name = alloc.memorylocations[0].name
        if alloc.kind == "ExternalInput":
            if name != partition_name:
                in_names.append(name)
        elif alloc.kind == "ExternalOutput":
            out_names.append(name)
            shape = tuple(alloc.tensor_shape)
            dtype = mybir.dt.np(alloc.dtype)
            out_avals.append(jax.core.ShapedArray(shape, dtype))
            zero_outs.append((shape, dtype))
    n_params = len(in_names)
    all_names = in_names + out_names
    if partition_name is not None:
        all_names = all_names + [partition_name]
    donate = tuple(range(n_params, n_params + len(out_names)))

    def _body(*args):
        operands = list(args)
        if partition_name is not None:
            operands.append(bass2jax.partition_id_tensor())
        outs = bass2jax._bass_exec_p.bind(
            *operands,
            out_avals=tuple(out_avals),
            in_names=tuple(all_names),
            out_names=tuple(out_names),
            lowering_input_output_aliases=(),
            sim_require_finite=True,
            sim_require_nnan=True,
            nc=nc,
        )
        return tuple(outs)

    devices = jax.devices()[:NCORES]
    mesh = Mesh(np.asarray(devices), ("core",))
    in_specs = tuple(
        PartitionSpec() if name in SHARED_INPUTS else PartitionSpec("core")
        for name in in_names
    ) + tuple(PartitionSpec("core") for _ in out_names)
    out_specs = tuple(PartitionSpec("core") for _ in out_names)
    fn = jax.jit(
        shard_map(_body, mesh=mesh, in_specs=in_specs, out_specs=out_specs,
                  check_rep=False),
        donate_argnums=donate,
        keep_unused=True,
    )
    return fn, mesh, in_names, out_names, zero_outs


def run_layer(runner, feed, time_exec=False):
    """feed: dict name -> np array (shared: local shape; per-core: stacked).
    Returns list of per-core outputs (+ measured ns when time_exec)."""
    import jax
    from jax.sharding import NamedSharding, PartitionSpec

    fn, mesh, in_names, out_names, zero_outs = runner
    args = []
    for name in in_names:
        spec = PartitionSpec() if name in SHARED_INPUTS else PartitionSpec("core")
        arr = feed[name]
        args.append(jax.device_put(arr, NamedSharding(mesh, spec)))
    zs = [
        jax.device_put(
            np.zeros((NCORES * s[0], *s[1:]), d),
            NamedSharding(mesh, PartitionSpec("core")),
        )
        for s, d in zero_outs
    ]
    outs = fn(*args, *zs)
    jax.block_until_ready(outs)
    best_ns = None
    if time_exec:
        import time
        for _ in range(3):
            zs = [
                jax.device_put(
                    np.zeros((NCORES * s[0], *s[1:]), d),
                    NamedSharding(mesh, PartitionSpec("core")),
                )
                for s, d in zero_outs
            ]
            t0 = time.perf_counter()
            outs2 = fn(*args, *zs)
            jax.block_until_ready(outs2)
            dt_ns = (time.perf_counter() - t0) * 1e9
            best_ns = dt_ns if best_ns is None else min(best_ns, dt_ns)
            outs = outs2
    res = {}
    for i, name in enumerate(out_names):
        a = np.asarray(outs[i])
        res[name] = a.reshape(NCORES, a.shape[0] // NCORES, *a.shape[1:])
    return res, best_ns


_cache = {}
last_hw_exec_ns = None
TIME_EXEC = False


def kernel(x, edge_index, W1, b1, W2, b2):
    global last_hw_exec_ns
    x = np.asarray(x)
    edge_index = np.asarray(edge_index)
    n = x.shape[0]
    cfg = Cfg(n, x.shape[1], np.asarray(W1).shape[1], np.asarray(W2).shape[1])
    st = prep(cfg, edge_index)

    key = st.struct_key
    if key not in _cache:
        nc1 = build_layer(cfg, st, cfg.F, cfg.H, relu=True, out_f32=False)
        nc2 = build_layer(cfg, st, cfg.H, cfg.C, relu=False, out_f32=True)
        _cache[key] = (make_runner(nc1), make_runner(nc2))
    r1, r2 = _cache[key]

    xT = np.zeros((cfg.F, cfg.NPAD), np.float32)
    xT[:, :n] = x.astype(np.float32).T
    feed1 = {
        "xT": xT.astype(BF16),
        "W": np.asarray(W1, np.float32).astype(BF16),
        "bias": np.asarray(b1, np.float32).reshape(-1, 1),
        "iota": st.iota,
        "idx16": st.idx16.reshape(NCORES * 16, st.LWT),
        "dstl": st.dstl.reshape(NCORES * P, st.CT),
        "norm": st.norm.reshape(NCORES * P, st.CT),
    }
    out1, ns1 = run_layer(r1, feed1, time_exec=TIME_EXEC)
    z1 = out1["zT"]  # [8, H, NB*128]

    NPOS = cfg.NBT * P
    z1_all = np.zeros((cfg.H, NPOS), np.float32)
    for c in range(NCORES):
        zc = z1[c].astype(np.float32)
        for s in range(cfg.NB):
            b = 8 * s + (c if s % 2 == 0 else 7 - c)
            z1_all[:, b * P:(b + 1) * P] = zc[:, s * P:(s + 1) * P]
    h_actT = np.zeros((cfg.H, cfg.NPAD), np.float32)
    h_actT[:, st.order] = z1_all[:, :n]

    feed2 = dict(feed1)
    feed2["xT"] = h_actT.astype(BF16)
    feed2["W"] = np.asarray(W2, np.float32).astype(BF16)
    feed2["bias"] = np.asarray(b2, np.float32).reshape(-1, 1)
    out2, ns2 = run_layer(r2, feed2, time_exec=TIME_EXEC)
    z2 = out2["zT"]

    out = np.zeros((n, cfg.C), np.float32)
    z2_all = np.zeros((cfg.C, NPOS), np.float32)
    for c in range(NCORES):
        zc = z2[c]
        for s in range(cfg.NB):
            b = 8 * s + (c if s % 2 == 0 else 7 - c)
            z2_all[:, b * P:(b + 1) * P] = zc[:, s * P:(s + 1) * P]
    out[st.order] = z2_all[:, :n].T
    if ns1 is not None and ns2 is not None:
        last_hw_exec_ns = int(ns1 + ns2)
    return out

